# revision 9
# baseline (speedup 1.0000x reference)
"""Multi-head attention Trainium2 kernel.

Problem: B=2, S=4096, D=512, H=8 heads, dk=dv=64 (fp32).
Sharding: head-parallel — core c computes head c for both batches.

Per-core algorithm (head h):
  Phase A: stream x in 512-token groups; PE-transpose to x^T; fp32r
    projections Q^T,K^T (transposed layout, dk on partitions) and V
    (natural layout, augmented with a ones column per 128-key block).
  Phase B: per 512-query group, per 128-key block:
    S^T = K^T_blk.T @ Q^T (row-packed: batch 0 in PE rows 0-63,
    batch 1 in rows 64-127, concurrent); ACT exp((S^T)/8) from PSUM in
    multi-bank groups -> P^T (fp32r); PV: [V|1].T @ P^T accumulates
    [65, 512] (row 64 = softmax denominators).
  Output: PE-transpose [65,128] blocks back to natural layout,
    normalize by 1/denominator (per-partition scalar), add bv, DMA out.
"""

import sys

sys.path.insert(0, "/opt/trn_rl_repo")

import numpy as np

import concourse.bass as bass
import concourse.tile as tile
from concourse import bacc, mybir
from concourse.bass_utils import run_bass_kernel_spmd

FP32 = mybir.dt.float32
FP32R = mybir.dt.float32r

B = 2
S = 4096
D = 512
DK = 64
HEADS = 8
N_CORES = 8

TG = 512          # tokens per phase-A group
QG = 512          # queries per phase-B group
KB = 128          # keys per block
GRP_A = 3         # score psum banks per ACT group, batch 0
GRP_B = 3         # score psum banks per ACT group, batch 1


def build_nc(s=S, reps=1):
    """Build the per-core Bass program (SPMD, same NEFF on all cores)."""
    toks = B * s             # total tokens
    n_tg = toks // TG        # phase-A groups
    n_qg = s // QG           # phase-B query groups per batch
    n_kb = s // KB           # key blocks per batch
    scale = 1.0 / np.sqrt(np.float64(DK))

    nc = bacc.Bacc("TRN2", target_bir_lowering=False, debug=False,
                   num_devices=N_CORES)

    x_d = nc.dram_tensor("x", [toks, D], FP32, kind="ExternalInput")
    wq_d = nc.dram_tensor("wq", [D, DK], FP32, kind="ExternalInput")
    wk_d = nc.dram_tensor("wk", [D, DK], FP32, kind="ExternalInput")
    wv_d = nc.dram_tensor("wv", [D, DK], FP32, kind="ExternalInput")
    bq_d = nc.dram_tensor("bq", [DK, 1], FP32, kind="ExternalInput")
    bk_d = nc.dram_tensor("bk", [DK, 1], FP32, kind="ExternalInput")
    bv4_d = nc.dram_tensor("bv4", [128, 256], FP32, kind="ExternalInput")
    id_d = nc.dram_tensor("ident", [128, 128], FP32, kind="ExternalInput")
    out_d = nc.dram_tensor("out", [toks, DK], FP32, kind="ExternalOutput")

    with tile.TileContext(nc) as tc:
        with tc.tile_pool(name="persist", bufs=1) as pp:
            ident = pp.tile([128, 128], FP32, tag="ident")
            nc.sync.dma_start(ident[:], id_d[:])
            wq_n = pp.tile([128, 256], FP32, tag="wq_n")
            wk_n = pp.tile([128, 256], FP32, tag="wk_n")
            wv_n = pp.tile([128, 256], FP32, tag="wv_n")
            # [512, 64] -> [128 part, 4 chunks x 64]
            for w_d, w_n in ((wq_d, wq_n), (wk_d, wk_n), (wv_d, wv_n)):
                nc.sync.dma_start(
                    w_n[:].rearrange("p (c m) -> p c m", m=64),
                    w_d.rearrange("(c p) m -> p c m", p=128))
            wq_r = pp.tile([128, 256], FP32R, tag="wq_r")
            wk_r = pp.tile([128, 256], FP32R, tag="wk_r")
            wv_r = pp.tile([128, 256], FP32R, tag="wv_r")
            nc.vector.tensor_copy(wq_r[:], wq_n[:])
            nc.vector.tensor_copy(wk_r[:], wk_n[:])
            nc.vector.tensor_copy(wv_r[:], wv_n[:])
            bq_t = pp.tile([DK, 1], FP32, tag="bq")
            bk_t = pp.tile([DK, 1], FP32, tag="bk")
            nc.sync.dma_start(bq_t[:], bq_d[:])
            nc.sync.dma_start(bk_t[:], bk_d[:])
            bv4_t = pp.tile([128, 256], FP32, tag="bv4")
            nc.sync.dma_start(bv4_t[:], bv4_d[:])

            # Q^T | K^T, both batches: rows 0:64 = batch 0, 64:128 = batch 1;
            # free: [0, s) = Q^T, [s, 2s) = K^T.
            qkt = pp.tile([128, 2 * s], FP32R, tag="qkt")
            # V augmented: per batch b, key-block g: cols (b*n_kb+g)*65 ..
            # +64 = V rows, col +64 = 1.0 (denominator column).
            v_sb = pp.tile([128, B * n_kb * 65], FP32R, tag="v_sb")
            v3 = v_sb[:].rearrange("p (g c) -> p g c", c=65)
            nc.vector.memset(v3[:, :, 64:65].bitcast(FP32), 1.0)

            for _rep in range(reps):
                _phases(nc, tc, s, n_tg, n_qg, n_kb, scale, x_d, out_d,
                        ident, wq_r, wk_r, wv_r, bq_t, bk_t, bv4_t, qkt, v3)
    nc.compile()
    return nc


def _phases(nc, tc, s, n_tg, n_qg, n_kb, scale, x_d, out_d,
            ident, wq_r, wk_r, wv_r, bq_t, bk_t, bv4_t, qkt, v3):
            # ---------------- Phase A ----------------
            with tc.tile_pool(name="pha_sb", bufs=3) as pa, \
                 tc.tile_pool(name="pha_vt_sb", bufs=2) as pvt, \
                 tc.tile_pool(name="pha_xt_ps", bufs=3, space="PSUM") as pxt, \
                 tc.tile_pool(name="pha_qk_ps", bufs=1, space="PSUM") as pqk, \
                 tc.tile_pool(name="pha_v_ps", bufs=1, space="PSUM") as pv_ps:
                for tg in range(n_tg):
                    b = tg // (n_tg // B)
                    toff = (tg % (n_tg // B)) * TG  # token offset in batch
                    rb = 64 * b                      # QKT row base
                    xnat = pa.tile([128, 2048], FP32, tag="xnat")
                    nc.sync.dma_start(
                        xnat[:].rearrange("p (a d) -> p a d", d=D),
                        x_d[tg * TG:(tg + 1) * TG, :]
                        .rearrange("(a p) d -> p a d", p=128))
                    xt = pa.tile([128, 2048], FP32R, tag="xt")
                    for c in range(4):
                        xtp = pxt.tile([128, 512], FP32, tag="xtp")
                        for a in range(4):
                            nc.tensor.transpose(
                                xtp[:, a * 128:(a + 1) * 128],
                                xnat[:, a * 512 + c * 128:
                                     a * 512 + (c + 1) * 128],
                                ident[:])
                        nc.vector.tensor_copy(
                            xt[:, c * 512:(c + 1) * 512], xtp[:])
                    # Q^T and K^T projections for this token group
                    psq = pqk.tile([DK, TG], FP32, tag="psq")
                    psk = pqk.tile([DK, TG], FP32, tag="psk")
                    for c in range(4):
                        nc.tensor.matmul(
                            psq[:], wq_r[:, c * 64:(c + 1) * 64],
                            xt[:, c * 512:(c + 1) * 512],
                            start=(c == 0), stop=(c == 3))
                    for c in range(4):
                        nc.tensor.matmul(
                            psk[:], wk_r[:, c * 64:(c + 1) * 64],
                            xt[:, c * 512:(c + 1) * 512],
                            start=(c == 0), stop=(c == 3))
                    nc.vector.tensor_scalar(
                        out=qkt[rb:rb + 64, toff:toff + TG], in0=psq[:],
                        scalar1=bq_t[:], scalar2=None,
                        op0=mybir.AluOpType.add)
                    nc.vector.tensor_scalar(
                        out=qkt[rb:rb + 64, s + toff:s + toff + TG],
                        in0=psk[:], scalar1=bk_t[:], scalar2=None,
                        op0=mybir.AluOpType.add)
                    # V^T projection, then PE-transpose to natural layout
                    psvt = pqk.tile([DK, TG], FP32, tag="psvt")
                    for c in range(4):
                        nc.tensor.matmul(
                            psvt[:], wv_r[:, c * 64:(c + 1) * 64],
                            xt[:, c * 512:(c + 1) * 512],
                            start=(c == 0), stop=(c == 3))
                    vt_sb = pvt.tile([DK, TG], FP32, tag="vt_sb")
                    nc.vector.tensor_copy(vt_sb[:], psvt[:])
                    vtr = pv_ps.tile([128, 256], FP32, tag="vtr")
                    for a in range(4):
                        nc.tensor.transpose(
                            vtr[:, a * 64:(a + 1) * 64],
                            vt_sb[:, a * 128:(a + 1) * 128],
                            ident[0:DK, 0:DK])
                    g0 = b * n_kb + (toff // 128)
                    nc.vector.tensor_copy(
                        v3[:, g0:g0 + 4, 0:64],
                        vtr[:].rearrange("p (a m) -> p a m", m=64))

            # ---------------- Phase B ----------------
            with tc.tile_pool(name="phb_sb", bufs=3) as pb, \
                 tc.tile_pool(name="phb_exp", bufs=3) as pexp, \
                 tc.tile_pool(name="phb_rcp", bufs=8) as prc, \
                 tc.tile_pool(name="phb_scA", bufs=1, space="PSUM") as pscA, \
                 tc.tile_pool(name="phb_scB", bufs=1, space="PSUM") as pscB, \
                 tc.tile_pool(name="phb_pv", bufs=2, space="PSUM") as ppv:
                def epilogue_tail(qg, augs):
                    # transposes + normalize + bias + store for a finished qg
                    # (deferred into the next qg so ACT's score pipeline
                    # refills first)
                    for b, aug in ((0, augs[0]), (1, augs[1])):
                        tr = ppv.tile([128, 4 * 65], FP32, tag="pv")
                        for a in range(4):
                            nc.tensor.transpose(
                                tr[:, a * 65:(a + 1) * 65],
                                aug[:, a * 128:(a + 1) * 128],
                                ident[0:65, 0:65])
                        onat = pb.tile([128, 256], FP32, tag="onat")
                        for a in range(4):
                            rcp = prc.tile([128, 1], FP32, tag="rcp")
                            nc.vector.reciprocal(
                                rcp[:], tr[:, a * 65 + 64:a * 65 + 65])
                            nc.vector.tensor_scalar(
                                out=onat[:, a * 64:(a + 1) * 64],
                                in0=tr[:, a * 65:a * 65 + 64],
                                scalar1=rcp[:], scalar2=None,
                                op0=mybir.AluOpType.mult)
                        ofin = pb.tile([128, 256], FP32, tag="ofin")
                        nc.vector.tensor_tensor(
                            out=ofin[:], in0=onat[:], in1=bv4_t[:],
                            op=mybir.AluOpType.add)
                        base = b * s + qg * QG
                        nc.sync.dma_start(
                            out_d[base:base + QG, :]
                            .rearrange("(a p) m -> p a m", p=128),
                            ofin[:].rearrange("p (a m) -> p a m", m=DK))

                pending = None  # (qg, (augA, augB)) awaiting tail
                for qg in range(n_qg):
                    pvA = ppv.tile([65, QG], FP32, tag="pv")
                    pvB = ppv.tile([65, QG], FP32, tag="pv")
                    psA = psB = None
                    pA = pB = None
                    for kb in range(n_kb):
                        ja = kb % GRP_A
                        jb = kb % GRP_B
                        la = min(GRP_A, n_kb - (kb - ja))  # group len A
                        lb = min(GRP_B, n_kb - (kb - jb))
                        if ja == 0:
                            psA = pscA.tile([128, GRP_A * 512], FP32,
                                            tag="scA")
                        if jb == 0:
                            psB = pscB.tile([128, GRP_B * 512], FP32,
                                            tag="scB")
                        # scores^T, row-packed across batches
                        nc.tensor.matmul(
                            psA[:, ja * 512:(ja + 1) * 512],
                            qkt[0:64, s + kb * 128:s + (kb + 1) * 128],
                            qkt[0:64, qg * QG:(qg + 1) * QG],
                            start=True, stop=True, tile_position=(0, 0))
                        nc.tensor.matmul(
                            psB[:, jb * 512:(jb + 1) * 512],
                            qkt[64:128, s + kb * 128:s + (kb + 1) * 128],
                            qkt[64:128, qg * QG:(qg + 1) * QG],
                            start=True, stop=True, tile_position=(64, 0))
                        if ja == la - 1:
                            pA = pexp.tile([128, GRP_A * 512], FP32R, tag="pA")
                            nc.scalar.activation(
                                pA[:, 0:la * 512], psA[:, 0:la * 512],
                                mybir.ActivationFunctionType.Exp,
                                scale=float(scale))
                        if jb == lb - 1:
                            pB = pexp.tile([128, GRP_B * 512], FP32R, tag="pB")
                            nc.scalar.activation(
                                pB[:, 0:lb * 512], psB[:, 0:lb * 512],
                                mybir.ActivationFunctionType.Exp,
                                scale=float(scale))
                        # PV for any kb whose exp group just completed
                        if ja == la - 1:
                            for j2 in range(la):
                                kb2 = kb - la + 1 + j2
                                nc.tensor.matmul(
                                    pvA[:], v3[:, kb2, :],
                                    pA[:, j2 * 512:(j2 + 1) * 512],
                                    start=(kb2 == 0), stop=(kb2 == n_kb - 1))
                        if jb == lb - 1:
                            for j2 in range(lb):
                                kb2 = kb - lb + 1 + j2
                                nc.tensor.matmul(
                                    pvB[:], v3[:, n_kb + kb2, :],
                                    pB[:, j2 * 512:(j2 + 1) * 512],
                                    start=(kb2 == 0), stop=(kb2 == n_kb - 1))
                        if kb == GRP_A - 1 and pending is not None:
                            epilogue_tail(*pending)
                            pending = None
                    # drain PV psum to SBUF now (frees the pv slots); defer
                    # the rest of the epilogue into the next qg
                    augA = pb.tile([65, QG], FP32, tag="aug")
                    nc.vector.tensor_copy(augA[:], pvA[:])
                    augB = pb.tile([65, QG], FP32, tag="aug")
                    nc.vector.tensor_copy(augB[:], pvB[:])
                    pending = (qg, (augA, augB))
                epilogue_tail(*pending)


_NC_CACHE = {}


def _get_nc(s=S, reps=1):
    key = (s, reps)
    if key not in _NC_CACHE:
        _NC_CACHE[key] = build_nc(s, reps)
    return _NC_CACHE[key]


def make_in_maps(inputs, s=S):
    x = np.ascontiguousarray(np.asarray(inputs["x"], dtype=np.float32))
    toks = B * s
    x_flat = x.reshape(toks, D)
    Wq = np.asarray(inputs["Wq"], dtype=np.float32)
    Wk = np.asarray(inputs["Wk"], dtype=np.float32)
    Wv = np.asarray(inputs["Wv"], dtype=np.float32)
    bq = np.asarray(inputs["bq"], dtype=np.float32)
    bk = np.asarray(inputs["bk"], dtype=np.float32)
    bv = np.asarray(inputs["bv"], dtype=np.float32)
    ident = np.eye(128, dtype=np.float32)
    in_maps = []
    for h in range(N_CORES):
        in_maps.append({
            "x": x_flat,
            "wq": np.ascontiguousarray(Wq[h]),
            "wk": np.ascontiguousarray(Wk[h]),
            "wv": np.ascontiguousarray(Wv[h]),
            "bq": np.ascontiguousarray(bq[h].reshape(DK, 1)),
            "bk": np.ascontiguousarray(bk[h].reshape(DK, 1)),
            "bv4": np.ascontiguousarray(np.tile(bv[h], (128, 4))),
            "ident": ident,
        })
    return in_maps


def assemble(results, s=S):
    toks = B * s
    out = np.empty((toks, HEADS * DK), dtype=np.float32)
    for h in range(N_CORES):
        out[:, h * DK:(h + 1) * DK] = results[h]["out"]
    return out.reshape(B, s, HEADS * DK)


def kernel(**inputs):
    nc = _get_nc(S)
    res = run_bass_kernel_spmd(nc, make_in_maps(inputs, S),
                               core_ids=list(range(N_CORES)))
    return assemble(res.results, S)


# revision 10
# speedup vs baseline: 1.1244x; 1.1244x over previous
"""Multi-head attention Trainium2 kernel.

Problem: B=2, S=4096, D=512, H=8 heads, dk=dv=64 (fp32).
Sharding: head-parallel — core c computes head c for both batches.

Per-core algorithm (head h):
  Phase A: stream x in 512-token groups; PE-transpose to x^T; fp32r
    projections Q^T,K^T (transposed layout, dk on partitions) and V
    (natural layout, augmented with a ones column per 128-key block).
  Phase B: per 512-query group, per 128-key block:
    S^T = K^T_blk.T @ Q^T (row-packed: batch 0 in PE rows 0-63,
    batch 1 in rows 64-127, concurrent); ACT exp((S^T)/8) from PSUM in
    multi-bank groups -> P^T (fp32r); PV: [V|1].T @ P^T accumulates
    [65, 512] (row 64 = softmax denominators).
  Output: PE-transpose [65,128] blocks back to natural layout,
    normalize by 1/denominator (per-partition scalar), add bv, DMA out.
"""

import sys

sys.path.insert(0, "/opt/trn_rl_repo")

import numpy as np

import concourse.bass as bass
import concourse.tile as tile
from concourse import bacc, mybir
from concourse.bass_utils import run_bass_kernel_spmd

FP32 = mybir.dt.float32
FP32R = mybir.dt.float32r

B = 2
S = 4096
D = 512
DK = 64
HEADS = 8
N_CORES = 8

TG = 512          # tokens per phase-A group
QG = 512          # queries per phase-B group
KB = 128          # keys per block
GRP_A = 3         # score psum banks per ACT group, batch 0
GRP_B = 3         # score psum banks per ACT group, batch 1


def build_nc(s=S, reps=1, phases="AB"):
    """Build the per-core Bass program (SPMD, same NEFF on all cores)."""
    toks = B * s             # total tokens
    n_tg = toks // TG        # phase-A groups
    n_qg = s // QG           # phase-B query groups per batch
    n_kb = s // KB           # key blocks per batch
    scale = 1.0 / np.sqrt(np.float64(DK))

    nc = bacc.Bacc("TRN2", target_bir_lowering=False, debug=False,
                   num_devices=N_CORES)

    x_d = nc.dram_tensor("x", [toks, D], FP32, kind="ExternalInput")
    wq_d = nc.dram_tensor("wq", [D, DK], FP32, kind="ExternalInput")
    wk_d = nc.dram_tensor("wk", [D, DK], FP32, kind="ExternalInput")
    wv_d = nc.dram_tensor("wv", [D, DK], FP32, kind="ExternalInput")
    bq_d = nc.dram_tensor("bq", [DK, 1], FP32, kind="ExternalInput")
    bk_d = nc.dram_tensor("bk", [DK, 1], FP32, kind="ExternalInput")
    bv4_d = nc.dram_tensor("bv4", [128, 256], FP32, kind="ExternalInput")
    id_d = nc.dram_tensor("ident", [128, 128], FP32, kind="ExternalInput")
    out_d = nc.dram_tensor("out", [toks, DK], FP32, kind="ExternalOutput")

    with tile.TileContext(nc) as tc:
        with tc.tile_pool(name="persist", bufs=1) as pp:
            ident = pp.tile([128, 128], FP32, tag="ident")
            nc.sync.dma_start(ident[:], id_d[:])
            wq_n = pp.tile([128, 256], FP32, tag="wq_n")
            wk_n = pp.tile([128, 256], FP32, tag="wk_n")
            wv_n = pp.tile([128, 256], FP32, tag="wv_n")
            # [512, 64] -> [128 part, 4 chunks x 64]
            for w_d, w_n in ((wq_d, wq_n), (wk_d, wk_n), (wv_d, wv_n)):
                nc.sync.dma_start(
                    w_n[:].rearrange("p (c m) -> p c m", m=64),
                    w_d.rearrange("(c p) m -> p c m", p=128))
            wq_r = pp.tile([128, 256], FP32R, tag="wq_r")
            wk_r = pp.tile([128, 256], FP32R, tag="wk_r")
            wv_r = pp.tile([128, 256], FP32R, tag="wv_r")
            nc.vector.tensor_copy(wq_r[:], wq_n[:])
            nc.vector.tensor_copy(wk_r[:], wk_n[:])
            nc.vector.tensor_copy(wv_r[:], wv_n[:])
            bq_t = pp.tile([DK, 1], FP32, tag="bq")
            bk_t = pp.tile([DK, 1], FP32, tag="bk")
            nc.sync.dma_start(bq_t[:], bq_d[:])
            nc.sync.dma_start(bk_t[:], bk_d[:])
            bv4_t = pp.tile([128, 256], FP32, tag="bv4")
            nc.sync.dma_start(bv4_t[:], bv4_d[:])

            # Q^T | K^T, both batches: rows 0:64 = batch 0, 64:128 = batch 1;
            # free: [0, s) = Q^T, [s, 2s) = K^T.
            qkt = pp.tile([128, 2 * s], FP32R, tag="qkt")
            # V augmented: per batch b, key-block g: cols (b*n_kb+g)*65 ..
            # +64 = V rows, col +64 = 1.0 (denominator column).
            v_sb = pp.tile([128, B * n_kb * 65], FP32R, tag="v_sb")
            v3 = v_sb[:].rearrange("p (g c) -> p g c", c=65)
            nc.vector.memset(v3[:, :, 64:65].bitcast(FP32), 1.0)

            for _rep in range(reps):
                _phases(nc, tc, s, n_tg, n_qg, n_kb, scale, x_d, out_d,
                        ident, wq_r, wk_r, wv_r, bq_t, bk_t, bv4_t, qkt, v3,
                        phases)
    nc.compile()
    return nc


def _phases(nc, tc, s, n_tg, n_qg, n_kb, scale, x_d, out_d,
            ident, wq_r, wk_r, wv_r, bq_t, bk_t, bv4_t, qkt, v3,
            phases="AB"):
            # ---------------- Phase A ----------------
            with tc.tile_pool(name="pha_sb", bufs=3) as pa, \
                 tc.tile_pool(name="pha_vt_sb", bufs=2) as pvt, \
                 tc.tile_pool(name="pha_xt_ps", bufs=3, space="PSUM") as pxt, \
                 tc.tile_pool(name="pha_qk_ps", bufs=1, space="PSUM") as pqk, \
                 tc.tile_pool(name="pha_v_ps", bufs=1, space="PSUM") as pv_ps:
                for tg in range(n_tg):
                    b = tg // (n_tg // B)
                    toff = (tg % (n_tg // B)) * TG  # token offset in batch
                    rb = 64 * b                      # QKT row base
                    xnat = pa.tile([128, 2048], FP32, tag="xnat")
                    nc.sync.dma_start(
                        xnat[:].rearrange("p (a d) -> p a d", d=D),
                        x_d[tg * TG:(tg + 1) * TG, :]
                        .rearrange("(a p) d -> p a d", p=128))
                    xt = pa.tile([128, 2048], FP32R, tag="xt")
                    for c in range(4):
                        xtp = pxt.tile([128, 512], FP32, tag="xtp")
                        for a in range(4):
                            nc.tensor.transpose(
                                xtp[:, a * 128:(a + 1) * 128],
                                xnat[:, a * 512 + c * 128:
                                     a * 512 + (c + 1) * 128],
                                ident[:])
                        nc.vector.tensor_copy(
                            xt[:, c * 512:(c + 1) * 512], xtp[:])
                    # Q^T and K^T projections for this token group
                    psq = pqk.tile([DK, TG], FP32, tag="psq")
                    psk = pqk.tile([DK, TG], FP32, tag="psk")
                    for c in range(4):
                        nc.tensor.matmul(
                            psq[:], wq_r[:, c * 64:(c + 1) * 64],
                            xt[:, c * 512:(c + 1) * 512],
                            start=(c == 0), stop=(c == 3))
                    for c in range(4):
                        nc.tensor.matmul(
                            psk[:], wk_r[:, c * 64:(c + 1) * 64],
                            xt[:, c * 512:(c + 1) * 512],
                            start=(c == 0), stop=(c == 3))
                    nc.vector.tensor_scalar(
                        out=qkt[rb:rb + 64, toff:toff + TG], in0=psq[:],
                        scalar1=bq_t[:], scalar2=None,
                        op0=mybir.AluOpType.add)
                    nc.vector.tensor_scalar(
                        out=qkt[rb:rb + 64, s + toff:s + toff + TG],
                        in0=psk[:], scalar1=bk_t[:], scalar2=None,
                        op0=mybir.AluOpType.add)
                    # V^T projection, then PE-transpose to natural layout
                    psvt = pqk.tile([DK, TG], FP32, tag="psvt")
                    for c in range(4):
                        nc.tensor.matmul(
                            psvt[:], wv_r[:, c * 64:(c + 1) * 64],
                            xt[:, c * 512:(c + 1) * 512],
                            start=(c == 0), stop=(c == 3))
                    vt_sb = pvt.tile([DK, TG], FP32, tag="vt_sb")
                    nc.vector.tensor_copy(vt_sb[:], psvt[:])
                    vtr = pv_ps.tile([128, 256], FP32, tag="vtr")
                    for a in range(4):
                        nc.tensor.transpose(
                            vtr[:, a * 64:(a + 1) * 64],
                            vt_sb[:, a * 128:(a + 1) * 128],
                            ident[0:DK, 0:DK])
                    g0 = b * n_kb + (toff // 128)
                    nc.vector.tensor_copy(
                        v3[:, g0:g0 + 4, 0:64],
                        vtr[:].rearrange("p (a m) -> p a m", m=64))

            # ---------------- Phase B ----------------
            if "B" not in phases:
                return
            with tc.tile_pool(name="phb_sb", bufs=3) as pb, \
                 tc.tile_pool(name="phb_exp", bufs=3) as pexp, \
                 tc.tile_pool(name="phb_rcp", bufs=8) as prc, \
                 tc.tile_pool(name="phb_scA", bufs=1, space="PSUM") as pscA, \
                 tc.tile_pool(name="phb_scB", bufs=1, space="PSUM") as pscB, \
                 tc.tile_pool(name="phb_pv", bufs=2, space="PSUM") as ppv:
                def epilogue_tail(qg, augs):
                    # transposes + normalize + bias + store for a finished qg
                    # (deferred into the next qg so ACT's score pipeline
                    # refills first)
                    for b, aug in ((0, augs[0]), (1, augs[1])):
                        tr = ppv.tile([128, 4 * 65], FP32, tag="pv")
                        for a in range(4):
                            nc.tensor.transpose(
                                tr[:, a * 65:(a + 1) * 65],
                                aug[:, a * 128:(a + 1) * 128],
                                ident[0:65, 0:65])
                        onat = pb.tile([128, 256], FP32, tag="onat")
                        for a in range(4):
                            rcp = prc.tile([128, 1], FP32, tag="rcp")
                            nc.vector.reciprocal(
                                rcp[:], tr[:, a * 65 + 64:a * 65 + 65])
                            nc.vector.tensor_scalar(
                                out=onat[:, a * 64:(a + 1) * 64],
                                in0=tr[:, a * 65:a * 65 + 64],
                                scalar1=rcp[:], scalar2=None,
                                op0=mybir.AluOpType.mult)
                        ofin = pb.tile([128, 256], FP32, tag="ofin")
                        nc.vector.tensor_tensor(
                            out=ofin[:], in0=onat[:], in1=bv4_t[:],
                            op=mybir.AluOpType.add)
                        base = b * s + qg * QG
                        nc.sync.dma_start(
                            out_d[base:base + QG, :]
                            .rearrange("(a p) m -> p a m", p=128),
                            ofin[:].rearrange("p (a m) -> p a m", m=DK))

                pending = None  # (qg, (augA, augB)) awaiting tail
                for qg in range(n_qg):
                    pvA = ppv.tile([65, QG], FP32, tag="pv")
                    pvB = ppv.tile([65, QG], FP32, tag="pv")
                    psA = psB = None
                    pA = pB = None
                    for kb in range(n_kb):
                        ja = kb % GRP_A
                        jb = kb % GRP_B
                        la = min(GRP_A, n_kb - (kb - ja))  # group len A
                        lb = min(GRP_B, n_kb - (kb - jb))
                        if ja == 0:
                            psA = pscA.tile([128, GRP_A * 512], FP32,
                                            tag="scA")
                        if jb == 0:
                            psB = pscB.tile([128, GRP_B * 512], FP32,
                                            tag="scB")
                        # scores^T, row-packed across batches
                        nc.tensor.matmul(
                            psA[:, ja * 512:(ja + 1) * 512],
                            qkt[0:64, s + kb * 128:s + (kb + 1) * 128],
                            qkt[0:64, qg * QG:(qg + 1) * QG],
                            start=True, stop=True, tile_position=(0, 0))
                        nc.tensor.matmul(
                            psB[:, jb * 512:(jb + 1) * 512],
                            qkt[64:128, s + kb * 128:s + (kb + 1) * 128],
                            qkt[64:128, qg * QG:(qg + 1) * QG],
                            start=True, stop=True, tile_position=(64, 0))
                        if ja == la - 1:
                            pA = pexp.tile([128, GRP_A * 512], FP32R, tag="pA")
                            nc.scalar.activation(
                                pA[:, 0:la * 512], psA[:, 0:la * 512],
                                mybir.ActivationFunctionType.Exp,
                                scale=float(scale))
                        if jb == lb - 1:
                            pB = pexp.tile([128, GRP_B * 512], FP32R, tag="pB")
                            nc.scalar.activation(
                                pB[:, 0:lb * 512], psB[:, 0:lb * 512],
                                mybir.ActivationFunctionType.Exp,
                                scale=float(scale))
                        # PV for any kb whose exp group just completed
                        if ja == la - 1:
                            for j2 in range(la):
                                kb2 = kb - la + 1 + j2
                                nc.tensor.matmul(
                                    pvA[:], v3[:, kb2, :],
                                    pA[:, j2 * 512:(j2 + 1) * 512],
                                    start=(kb2 == 0), stop=(kb2 == n_kb - 1))
                        if jb == lb - 1:
                            for j2 in range(lb):
                                kb2 = kb - lb + 1 + j2
                                nc.tensor.matmul(
                                    pvB[:], v3[:, n_kb + kb2, :],
                                    pB[:, j2 * 512:(j2 + 1) * 512],
                                    start=(kb2 == 0), stop=(kb2 == n_kb - 1))
                        if kb == GRP_A - 1 and pending is not None:
                            epilogue_tail(*pending)
                            pending = None
                    # drain PV psum to SBUF now (frees the pv slots); defer
                    # the rest of the epilogue into the next qg
                    augA = pb.tile([65, QG], FP32, tag="aug")
                    nc.vector.tensor_copy(augA[:], pvA[:])
                    augB = pb.tile([65, QG], FP32, tag="aug")
                    nc.vector.tensor_copy(augB[:], pvB[:])
                    pending = (qg, (augA, augB))
                epilogue_tail(*pending)


_NC_CACHE = {}


def _get_nc(s=S, reps=1, phases="AB"):
    key = (s, reps, phases)
    if key not in _NC_CACHE:
        _NC_CACHE[key] = build_nc(s, reps, phases)
    return _NC_CACHE[key]


def make_in_maps(inputs, s=S):
    x = np.ascontiguousarray(np.asarray(inputs["x"], dtype=np.float32))
    toks = B * s
    x_flat = x.reshape(toks, D)
    Wq = np.asarray(inputs["Wq"], dtype=np.float32)
    Wk = np.asarray(inputs["Wk"], dtype=np.float32)
    Wv = np.asarray(inputs["Wv"], dtype=np.float32)
    bq = np.asarray(inputs["bq"], dtype=np.float32)
    bk = np.asarray(inputs["bk"], dtype=np.float32)
    bv = np.asarray(inputs["bv"], dtype=np.float32)
    ident = np.eye(128, dtype=np.float32)
    in_maps = []
    for h in range(N_CORES):
        in_maps.append({
            "x": x_flat,
            "wq": np.ascontiguousarray(Wq[h]),
            "wk": np.ascontiguousarray(Wk[h]),
            "wv": np.ascontiguousarray(Wv[h]),
            "bq": np.ascontiguousarray(bq[h].reshape(DK, 1)),
            "bk": np.ascontiguousarray(bk[h].reshape(DK, 1)),
            "bv4": np.ascontiguousarray(np.tile(bv[h], (128, 4))),
            "ident": ident,
        })
    return in_maps


def assemble(results, s=S):
    toks = B * s
    out = np.empty((toks, HEADS * DK), dtype=np.float32)
    for h in range(N_CORES):
        out[:, h * DK:(h + 1) * DK] = results[h]["out"]
    return out.reshape(B, s, HEADS * DK)


def kernel(**inputs):
    nc = _get_nc(S)
    res = run_bass_kernel_spmd(nc, make_in_maps(inputs, S),
                               core_ids=list(range(N_CORES)))
    return assemble(res.results, S)
